# revision 4
# baseline (speedup 1.0000x reference)
"""ResNet BasicBlock forward on 8 Trainium2 NeuronCores.

Computes relu(bn2(conv2(relu(bn1(conv1(x))))) + x) for x[64,128,56,56],
two 3x3 stride-1 pad-1 convs with 128->128 channels, eval-mode BN.

Strategy:
  - Data parallel over batch: 8 images per core, no collectives.
  - Each 3x3 conv = 9 shifted matmuls accumulated in PSUM. Input channels
    (128) sit on the SBUF partition dim (= matmul contraction dim); output
    channels land on PSUM partitions. Spatial output is tiled into 7 PSUM
    banks of 8 rows x 56 cols (448 fp32 = one 2KB bank).
  - bf16 matmul inputs (1 cycle/row on the PE vs 4 for fp32), fp32 PSUM
    accumulation. x is cast to bf16 on the host and DMAd straight into a
    58x58 zero-bordered SBUF buffer, so every shifted 3x3 tap is a plain
    strided access pattern and padding costs nothing. BN scale is folded
    into the conv weights on the host; BN shift is a per-channel bias.
  - Epilogues: ScalarE does relu(psum + b1) -> bf16 mid (padded); VectorE
    does (psum + b2) + residual then relu. The residual is read from the
    bf16 input buffer (adds <0.1% error, saves a 12.8MB/core fp32 stream).
  - Ramp/tail: image 0's input DMA is split into 7 row chunks so the first
    matmul starts as soon as ~2 chunks land; a dummy activation hoists the
    ACT table load off the critical path; the last image's output DMA is
    chunked per bank. The Tile framework inserts all semaphores; images
    are software-pipelined DEPTH deep.
"""

import functools
import os
import sys

import numpy as np

for _p in ("/opt/trn_rl_repo", "/root/.axon_site/_ro/trn_rl_repo"):
    if os.path.isdir(_p) and _p not in sys.path:
        sys.path.append(_p)

import ml_dtypes  # noqa: E402

import concourse.bass as bass  # noqa: E402,F401
import concourse.mybir as mybir  # noqa: E402
import concourse.tile as tile  # noqa: E402
from concourse import bacc, bass_utils  # noqa: E402

N_CORES = 8
IMGS = 8  # images per core
C = 128
H = W = 56
HP = WP = 58  # padded spatial
RPB = 8  # output rows per PSUM bank
BANKS = H // RPB  # 7
KK = 9  # 3x3 taps
EPS = 1e-5
DEPTH = 3  # image pipeline depth

BF16 = mybir.dt.bfloat16
F32 = mybir.dt.float32


def _build_module():
    nc = bacc.Bacc(
        "TRN2",
        target_bir_lowering=False,
        debug=False,
        enable_asserts=False,
        num_devices=N_CORES,
    )
    xbf_d = nc.dram_tensor("xbf", [IMGS, C, H, W], BF16, kind="ExternalInput").ap()
    w1_d = nc.dram_tensor("w1t", [C, KK, C], BF16, kind="ExternalInput").ap()
    w2_d = nc.dram_tensor("w2t", [C, KK, C], BF16, kind="ExternalInput").ap()
    b1_d = nc.dram_tensor("b1", [C, 1], F32, kind="ExternalInput").ap()
    b2_d = nc.dram_tensor("b2", [C, 1], F32, kind="ExternalInput").ap()
    out_d = nc.dram_tensor("out", [IMGS, C, H, W], F32, kind="ExternalOutput").ap()

    add = mybir.AluOpType.add
    relu = mybir.ActivationFunctionType.Relu

    with tile.TileContext(nc) as tc:
        with (
            tc.tile_pool(name="singles", bufs=1) as singles,
            tc.tile_pool(name="psum", bufs=8, space="PSUM") as psum_pool,
        ):
            w1_sb = singles.tile([C, KK, C], BF16, name="w1_sb")
            w2_sb = singles.tile([C, KK, C], BF16, name="w2_sb")
            b1_sb = singles.tile([C, 1], F32, name="b1_sb")
            b2_sb = singles.tile([C, 1], F32, name="b2_sb")
            dummy = singles.tile([C, 1], F32, name="dummy")
            warm = singles.tile([C, 448], BF16, name="warm")

            # Hoist the ACT table load off the critical path: first ACTIVATE
            # in the Scalar stream triggers it, so issue a dependency-free
            # dummy immediately.
            nc.vector.memset(dummy, 0.0)
            nc.vector.memset(warm, 0.0)
            nc.scalar.activation(out=dummy, in_=dummy, func=relu)

            x_pad = [
                singles.tile([C, HP, WP], BF16, name=f"x_pad{d}") for d in range(DEPTH)
            ]
            mid_pad = [
                singles.tile([C, HP, WP], BF16, name=f"mid_pad{d}")
                for d in range(DEPTH)
            ]
            out_sb = [
                singles.tile([C, H, W], F32, name=f"out_sb{d}") for d in range(DEPTH)
            ]

            # Warm up the PE's HAM clock gate while image 0's DMA is in
            # flight: ~12 throwaway matmuls keep the PE busy so the real
            # ones start at 2.4 GHz.
            wps = psum_pool.tile([C, 448], F32, name="ps")
            for wi in range(12):
                nc.tensor.matmul(
                    wps,
                    lhsT=warm[:, 0:C],
                    rhs=warm[:, :],
                    start=(wi == 0),
                    stop=(wi == 11),
                )

            # Image 0's input: w1 first on the sync HWDGE queue; row chunks
            # 0/1 in parallel on the scalar HWDGE queue so conv bank 0 can
            # start as soon as they land; remaining chunks follow on sync.
            nc.sync.dma_start(out=w1_sb, in_=w1_d)
            for cchunk in range(BANKS):
                eng = nc.scalar if cchunk < 2 else nc.sync
                eng.dma_start(
                    out=x_pad[0][:, 1 + RPB * cchunk : 1 + RPB * (cchunk + 1), 1 : W + 1],
                    in_=xbf_d[0][:, RPB * cchunk : RPB * (cchunk + 1), :],
                )

            nc.sync.dma_start(out=w2_sb, in_=w2_d)
            nc.sync.dma_start(out=b1_sb, in_=b1_d)
            nc.sync.dma_start(out=b2_sb, in_=b2_d)

            # Zero borders once (GpSimd, off the critical path): row 0,
            # row 57, and cols 0/57 of rows 1..56. Interiors are fully
            # rewritten per image.
            for buf in x_pad + mid_pad:
                nc.gpsimd.memset(buf[:, 0, :], 0.0)
                nc.gpsimd.memset(buf[:, HP - 1, :], 0.0)
                nc.gpsimd.memset(buf[:, 1 : HP - 1, 0 : WP : WP - 1], 0.0)

            for i in range(IMGS):
                d = i % DEPTH
                xp, mp, ob = x_pad[d], mid_pad[d], out_sb[d]
                if i > 0:
                    nc.sync.dma_start(
                        out=xp[:, 1 : H + 1, 1 : W + 1], in_=xbf_d[i]
                    )

                # conv1 + bn1 + relu -> mid (bf16, padded)
                for b in range(BANKS):
                    ps = psum_pool.tile([C, RPB, W], F32, name="ps")
                    for kk in range(KK):
                        ky, kx = divmod(kk, 3)
                        nc.tensor.matmul(
                            ps,
                            lhsT=w1_sb[:, kk, :],
                            rhs=xp[:, RPB * b + ky : RPB * b + ky + RPB, kx : kx + W],
                            start=(kk == 0),
                            stop=(kk == KK - 1),
                        )
                    nc.scalar.activation(
                        out=mp[:, 1 + RPB * b : 1 + RPB * (b + 1), 1 : W + 1],
                        in_=ps,
                        func=relu,
                        bias=b1_sb[:, 0:1],
                    )

                # conv2 + bn2 + residual + relu -> out
                for b in range(BANKS):
                    ps2 = psum_pool.tile([C, RPB, W], F32, name="ps")
                    for kk in range(KK):
                        ky, kx = divmod(kk, 3)
                        nc.tensor.matmul(
                            ps2,
                            lhsT=w2_sb[:, kk, :],
                            rhs=mp[:, RPB * b + ky : RPB * b + ky + RPB, kx : kx + W],
                            start=(kk == 0),
                            stop=(kk == KK - 1),
                        )
                    rows = ob[:, RPB * b : RPB * (b + 1), :]
                    nc.vector.scalar_tensor_tensor(
                        out=rows,
                        in0=ps2,
                        scalar=b2_sb[:, 0:1],
                        in1=xp[:, 1 + RPB * b : 1 + RPB * (b + 1), 1 : W + 1],
                        op0=add,
                        op1=add,
                    )
                    nc.vector.tensor_scalar_max(rows, rows, 0.0)
                    if i == IMGS - 1:
                        nc.scalar.dma_start(
                            out=out_d[i][:, RPB * b : RPB * (b + 1), :], in_=rows
                        )

                if i < IMGS - 1:
                    nc.scalar.dma_start(out=out_d[i], in_=ob)

    nc.compile()
    return nc


@functools.lru_cache(maxsize=1)
def _get_module():
    return _build_module()


def _prep_in_maps(inputs):
    f32 = np.float32
    x = np.asarray(inputs["x"], f32)
    w1 = np.asarray(inputs["w1"], f32)
    w2 = np.asarray(inputs["w2"], f32)
    gamma1 = np.asarray(inputs["gamma1"], f32)
    beta1 = np.asarray(inputs["beta1"], f32)
    mean1 = np.asarray(inputs["mean1"], f32)
    var1 = np.asarray(inputs["var1"], f32)
    gamma2 = np.asarray(inputs["gamma2"], f32)
    beta2 = np.asarray(inputs["beta2"], f32)
    mean2 = np.asarray(inputs["mean2"], f32)
    var2 = np.asarray(inputs["var2"], f32)

    a1 = gamma1 / np.sqrt(var1 + EPS)
    a2 = gamma2 / np.sqrt(var2 + EPS)
    # Fold BN scale into weights; transpose to [c_in, tap, c_out] for lhsT.
    w1t = np.ascontiguousarray(
        np.transpose(w1 * a1[:, None, None, None], (1, 2, 3, 0)).reshape(C, KK, C)
    ).astype(ml_dtypes.bfloat16)
    w2t = np.ascontiguousarray(
        np.transpose(w2 * a2[:, None, None, None], (1, 2, 3, 0)).reshape(C, KK, C)
    ).astype(ml_dtypes.bfloat16)
    b1 = np.ascontiguousarray((beta1 - mean1 * a1).reshape(C, 1).astype(f32))
    b2 = np.ascontiguousarray((beta2 - mean2 * a2).reshape(C, 1).astype(f32))

    xbf = np.ascontiguousarray(x).astype(ml_dtypes.bfloat16)
    return [
        {
            "xbf": xbf[IMGS * i : IMGS * (i + 1)],
            "w1t": w1t,
            "w2t": w2t,
            "b1": b1,
            "b2": b2,
        }
        for i in range(N_CORES)
    ]


def _run(inputs, trace=False):
    nc = _get_module()
    in_maps = _prep_in_maps(inputs)
    res = bass_utils.run_bass_kernel_spmd(
        nc, in_maps, core_ids=list(range(N_CORES)), trace=trace
    )
    out = np.concatenate([r["out"] for r in res.results], axis=0)
    return out.astype(np.float32), res


def kernel(**inputs):
    out, _ = _run(inputs, trace=False)
    return out
